# revision 1
# baseline (speedup 1.0000x reference)
"""Trainium2 Bass kernel for a 2-layer GAT (50k nodes, 1.6M+50k edges) on 8
NeuronCores.

Sharding: edges partitioned by dst range (6250/core), dst-sorted, packed into
98 windows of 64 dsts with a static per-window chunk quota so one compiled
SPMD program serves all cores; per-core variability lives in int32 index
tapes (prescaled element offsets into row-major DRAM feature tables).

Per 128-edge chunk: an indirect DMA gathers augmented feature rows
[h | as | ad] by src (one row per partition) plus the dst node's `ad` scalars
from a per-core dst table; ACT computes w = exp(leakyrelu(as+ad)); DVE folds
w into the features; a PSUM-accumulating matmul against a one-hot edge->dst
matrix aggregates numerators and per-head softmax denominators in one pass.
"""
import sys

for _p in ("/opt/trn_rl_repo",):
    if _p not in sys.path:
        sys.path.insert(0, _p)

import numpy as np
import ml_dtypes

import concourse.bass as bass
import concourse.bacc as bacc
import concourse.mybir as mybir
import concourse.tile as tile
from concourse.bass_utils import run_bass_kernel_spmd
from concourse.masks import make_identity

BF = mybir.dt.bfloat16
F32 = mybir.dt.float32
I32 = mybir.dt.int32

N = 50000
IN_CH = 128
HID = 32
H1 = 4
H2 = 2
OUT_CH = 3
NEG = 0.2

NCORES = 8
LOCAL = 6250
DW = 64
NWIN = 98
LPAD = NWIN * DW          # 6272
NPAD = NCORES * LPAD      # 50176
P = 128
SUBG = 16
R1 = 136                  # L1 table row elems: h(128) | as(4) | ad(4)
R2 = 68                   # L2 table row elems: h2(64) | as2(2) | ad2(2)


def _bf(a):
    return np.ascontiguousarray(np.asarray(a, np.float32)).astype(ml_dtypes.bfloat16)


def _blockdiag(a, heads, ch):
    m = np.zeros((heads * ch, heads), np.float32)
    for h in range(heads):
        m[h * ch:(h + 1) * ch, h] = np.asarray(a, np.float32)[h]
    return m


def _schedule(dst):
    counts = np.zeros((NCORES, NWIN), np.int64)
    per_core = []
    for k in range(NCORES):
        m = (dst >= k * LOCAL) & (dst < (k + 1) * LOCAL)
        idx = np.nonzero(m)[0]
        dl = (dst[idx] - k * LOCAL).astype(np.int64)
        w = dl // DW
        np.add.at(counts[k], (w,), 1)
        per_core.append((idx, dl, w))
    F = int(max(1, (counts.max() + P - 1) // P))
    return F, per_core


def _tapes(per_core_k, F, src_off, dst_off):
    idx, dl, w = per_core_k
    C = NWIN * F
    src_slots = np.zeros((C, P), np.int64)
    dst_slots = np.zeros((C, P), np.int64)
    dstl_slots = np.full((C, P), float(DW), np.float32)
    order = np.argsort(w, kind="stable")
    idx, dl, w = idx[order], dl[order], w[order]
    pos = np.zeros(len(idx), np.int64)
    uq, st = np.unique(w, return_index=True)
    ends = np.append(st[1:], len(w))
    for u, s0, s1 in zip(uq, st, ends):
        pos[s0:s1] = np.arange(s1 - s0)
    chunk = w * F + pos // P
    slot = pos % P
    src_slots[chunk, slot] = src_off[idx]
    dst_slots[chunk, slot] = dst_off[idx]
    dstl_slots[chunk, slot] = (dl % DW).astype(np.float32)
    return (np.ascontiguousarray(src_slots.T.astype(np.int32)),
            np.ascontiguousarray(dst_slots.T.astype(np.int32)),
            np.ascontiguousarray(dstl_slots.T.astype(np.float32)))


def _build_program(F1, F2):
    nc = bacc.Bacc("TRN2", target_bir_lowering=False, debug=False,
                   num_devices=NCORES)
    C1 = NWIN * F1
    C2 = NWIN * F2
    NT1 = NPAD // P   # 392
    NTD = LPAD // P   # 49

    xT = nc.dram_tensor("xT", [P, NPAD], BF, kind="ExternalInput")
    xdT = nc.dram_tensor("xdT", [P, LPAD], BF, kind="ExternalInput")
    w1aug = nc.dram_tensor("w1aug", [P, R1], BF, kind="ExternalInput")
    w2aug = nc.dram_tensor("w2aug", [P, R2], BF, kind="ExternalInput")
    wlin = nc.dram_tensor("wlin", [HID, OUT_CH], BF, kind="ExternalInput")
    blin = nc.dram_tensor("blin", [OUT_CH, 1], F32, kind="ExternalInput")
    b1r = nc.dram_tensor("b1r", [1, P], F32, kind="ExternalInput")
    b2r = nc.dram_tensor("b2r", [1, HID], F32, kind="ExternalInput")
    s1t = nc.dram_tensor("s1t", [P, C1], I32, kind="ExternalInput")
    d1t = nc.dram_tensor("d1t", [P, C1], I32, kind="ExternalInput")
    l1t = nc.dram_tensor("l1t", [P, C1], F32, kind="ExternalInput")
    s2t = nc.dram_tensor("s2t", [P, C2], I32, kind="ExternalInput")
    d2t = nc.dram_tensor("d2t", [P, C2], I32, kind="ExternalInput")
    l2t = nc.dram_tensor("l2t", [P, C2], F32, kind="ExternalInput")

    yt = nc.dram_tensor("yt", [OUT_CH, LPAD], F32, kind="ExternalOutput")

    t1g = nc.dram_tensor("t1g", [NPAD * R1], BF)
    t1d = nc.dram_tensor("t1d", [LPAD * R1], BF)
    t2loc = nc.dram_tensor("t2loc", [LPAD * R2], BF)
    t2full = nc.dram_tensor("t2full", [NCORES * LPAD * R2], BF,
                            addr_space="Shared")

    with tile.TileContext(nc) as tc:
        with tc.tile_pool(name="sb", bufs=3) as sb, \
             tc.tile_pool(name="cst", bufs=1) as cst, \
             tc.tile_pool(name="ps", bufs=2, space="PSUM") as ps:

            ident = cst.tile([P, P], BF)
            make_identity(nc, ident[:])
            iota_i = cst.tile([P, DW], mybir.dt.int32)
            nc.gpsimd.iota(iota_i[:], pattern=[[1, DW]], base=0,
                           channel_multiplier=0)
            iota_b = cst.tile([P, DW], BF)
            nc.vector.tensor_copy(out=iota_b[:], in_=iota_i[:])
            w1aug_t = cst.tile([P, R1], BF)
            nc.sync.dma_start(out=w1aug_t[:], in_=w1aug[:])
            w2aug_t = cst.tile([P, R2], BF)
            nc.sync.dma_start(out=w2aug_t[:], in_=w2aug[:])
            wlin_t = cst.tile([HID, OUT_CH], BF)
            nc.sync.dma_start(out=wlin_t[:], in_=wlin[:])
            blin_t = cst.tile([OUT_CH, 1], F32)
            nc.sync.dma_start(out=blin_t[:], in_=blin[:])
            ones1 = cst.tile([1, P], F32)
            nc.gpsimd.memset(ones1[:], 1.0)
            b1r_t = cst.tile([1, P], F32)
            nc.sync.dma_start(out=b1r_t[:], in_=b1r[:])
            b2r_t = cst.tile([1, HID], F32)
            nc.sync.dma_start(out=b2r_t[:], in_=b2r[:])
            brep_ps = ps.tile([P, P], F32, tag="sc")
            nc.tensor.matmul(out=brep_ps[:], lhsT=ones1[:], rhs=b1r_t[:],
                             start=True, stop=True)
            b1rep = cst.tile([P, P], F32)
            nc.vector.tensor_copy(out=b1rep[:], in_=brep_ps[:])
            brep2_ps = ps.tile([P, HID], F32, tag="sc")
            nc.tensor.matmul(out=brep2_ps[:], lhsT=ones1[:], rhs=b2r_t[:],
                             start=True, stop=True)
            b2rep = cst.tile([P, HID], F32)
            nc.vector.tensor_copy(out=b2rep[:], in_=brep2_ps[:])

            def stage_a(src_dram, ntiles, out_dram):
                for t in range(ntiles):
                    xs = sb.tile([P, P], BF, tag="xs")
                    nc.sync.dma_start(out=xs[:],
                                      in_=src_dram[:, t * P:(t + 1) * P])
                    hps = ps.tile([P, R1], F32, tag="hg")
                    nc.tensor.matmul(out=hps[:], lhsT=xs[:], rhs=w1aug_t[:],
                                     start=True, stop=True)
                    hsb = sb.tile([P, R1], BF, tag="hsb")
                    nc.scalar.copy(out=hsb[:], in_=hps[:])
                    nc.sync.dma_start(
                        out=out_dram[t * P * R1:(t + 1) * P * R1]
                            .rearrange("(p r) -> p r", r=R1),
                        in_=hsb[:])
            stage_a(xT, NT1, t1g)
            stage_a(xdT, NTD, t1d)

            def edge_layer(Fl, stape, dtape, ltape, tab, dtab, heads, hwid,
                           rowl, epilogue):
                rhw = hwid + heads
                for w in range(NWIN):
                    sidx = sb.tile([P, Fl], I32, tag="sidx")
                    nc.sync.dma_start(out=sidx[:],
                                      in_=stape[:, w * Fl:(w + 1) * Fl])
                    didx = sb.tile([P, Fl], I32, tag="didx")
                    nc.sync.dma_start(out=didx[:],
                                      in_=dtape[:, w * Fl:(w + 1) * Fl])
                    dstl = sb.tile([P, Fl], F32, tag="dstl")
                    nc.sync.dma_start(out=dstl[:],
                                      in_=ltape[:, w * Fl:(w + 1) * Fl])
                    dstl_b = sb.tile([P, Fl], BF, tag="dstlb")
                    nc.vector.tensor_copy(out=dstl_b[:], in_=dstl[:])
                    s_sb = sb.tile([P, Fl, DW], BF, tag="s_sb")
                    nc.vector.tensor_tensor(
                        out=s_sb[:],
                        in0=iota_b[:][:, None, :].to_broadcast([P, Fl, DW]),
                        in1=dstl_b[:][:, :, None].to_broadcast([P, Fl, DW]),
                        op=mybir.AluOpType.is_equal)

                    agg = ps.tile([DW, rhw], F32, tag="agg")
                    hs = sb.tile([P, Fl, rowl], BF, tag="hs")
                    ad = sb.tile([P, Fl, 4], BF, tag="ad")

                    for c in range(Fl):
                        nc.gpsimd.indirect_dma_start(
                            out=hs[:, c, 0:hwid + heads], out_offset=None,
                            in_=tab[:, None],
                            in_offset=bass.IndirectOffsetOnAxis(
                                ap=sidx[:, c:c + 1], axis=0))
                        nc.gpsimd.indirect_dma_start(
                            out=ad[:, c, 0:heads], out_offset=None,
                            in_=dtab[:, None],
                            in_offset=bass.IndirectOffsetOnAxis(
                                ap=didx[:, c:c + 1], axis=0))
                    for g0 in range(0, Fl, SUBG):
                        gn = min(SUBG, Fl - g0)
                        tt = sb.tile([P, SUBG, 4], F32, tag="tt")
                        nc.vector.tensor_tensor(
                            out=tt[:, :gn, 0:heads],
                            in0=hs[:, g0:g0 + gn, hwid:hwid + heads],
                            in1=ad[:, g0:g0 + gn, 0:heads],
                            op=mybir.AluOpType.add)
                        t2 = sb.tile([P, SUBG, 4], F32, tag="t2")
                        nc.vector.tensor_scalar_mul(t2[:, :gn, 0:heads],
                                                    tt[:, :gn, 0:heads], NEG)
                        nc.vector.tensor_tensor(
                            out=tt[:, :gn, 0:heads], in0=tt[:, :gn, 0:heads],
                            in1=t2[:, :gn, 0:heads], op=mybir.AluOpType.max)
                        wsb = sb.tile([P, SUBG, 4], F32, tag="wsb")
                        nc.scalar.activation(
                            out=wsb[:, :gn, 0:heads], in_=tt[:, :gn, 0:heads],
                            func=mybir.ActivationFunctionType.Exp)
                        wsb_b = sb.tile([P, SUBG, 4], BF, tag="wsbb")
                        nc.vector.tensor_copy(out=wsb_b[:, :gn, 0:heads],
                                              in_=wsb[:, :gn, 0:heads])
                        for j in range(gn):
                            c = g0 + j
                            nc.vector.tensor_tensor(
                                out=hs[:, c, 0:hwid].rearrange(
                                    "p (h f) -> p h f", h=heads),
                                in0=hs[:, c, 0:hwid].rearrange(
                                    "p (h f) -> p h f", h=heads),
                                in1=wsb_b[:, j, 0:heads][:, :, None]
                                    .to_broadcast([P, heads, hwid // heads]),
                                op=mybir.AluOpType.mult)
                        nc.vector.tensor_copy(
                            out=hs[:, g0:g0 + gn, hwid:hwid + heads],
                            in_=wsb_b[:, 0:gn, 0:heads])
                        for j in range(gn):
                            c = g0 + j
                            nc.tensor.matmul(out=agg[:], lhsT=s_sb[:, c, :],
                                             rhs=hs[:, c, 0:rhw],
                                             start=(c == 0), stop=(c == Fl - 1))
                    epilogue(w, agg)

            def epi1(w, agg):
                rec = sb.tile([DW, H1], F32, tag="rec")
                nc.vector.reciprocal(out=rec[:], in_=agg[:, P:P + H1])
                v = sb.tile([DW, P], F32, tag="v")
                nc.vector.tensor_tensor(
                    out=v[:].rearrange("d (h f) -> d h f", h=H1),
                    in0=agg[:, 0:P].rearrange("d (h f) -> d h f", h=H1),
                    in1=rec[:][:, :, None].to_broadcast([DW, H1, HID]),
                    op=mybir.AluOpType.mult)
                nc.vector.tensor_tensor(out=v[:], in0=v[:], in1=b1rep[:DW, :],
                                        op=mybir.AluOpType.add)
                m = sb.tile([DW, P], F32, tag="m")
                nc.vector.tensor_scalar_min(m[:], v[:], 0.0)
                nc.scalar.activation(out=m[:], in_=m[:],
                                     func=mybir.ActivationFunctionType.Exp)
                nc.vector.tensor_scalar_sub(m[:], m[:], 1.0)
                x2 = sb.tile([DW, P], BF, tag="x2")
                nc.vector.tensor_tensor(out=x2[:], in0=v[:], in1=m[:],
                                        op=mybir.AluOpType.max)
                x2T_ps = ps.tile([P, DW], BF, tag="sc")
                nc.tensor.transpose(out=x2T_ps[:], in_=x2[:],
                                    identity=ident[:DW, :DW])
                x2T = sb.tile([P, DW], BF, tag="x2T")
                nc.scalar.copy(out=x2T[:], in_=x2T_ps[:])
                h2_ps = ps.tile([DW, R2], F32, tag="hg")
                nc.tensor.matmul(out=h2_ps[:], lhsT=x2T[:], rhs=w2aug_t[:],
                                 start=True, stop=True)
                h2 = sb.tile([DW, R2], BF, tag="h2")
                nc.scalar.copy(out=h2[:], in_=h2_ps[:])
                nc.sync.dma_start(
                    out=t2loc[w * DW * R2:(w + 1) * DW * R2]
                        .rearrange("(p r) -> p r", r=R2),
                    in_=h2[:])

            def epi2(w, agg):
                rec = sb.tile([DW, H2], F32, tag="rec")
                nc.vector.reciprocal(out=rec[:], in_=agg[:, 64:64 + H2])
                v = sb.tile([DW, 64], F32, tag="v")
                nc.vector.tensor_tensor(
                    out=v[:].rearrange("d (h f) -> d h f", h=H2),
                    in0=agg[:, 0:64].rearrange("d (h f) -> d h f", h=H2),
                    in1=rec[:][:, :, None].to_broadcast([DW, H2, HID]),
                    op=mybir.AluOpType.mult)
                x3 = sb.tile([DW, HID], F32, tag="x3f")
                nc.vector.tensor_tensor(out=x3[:], in0=v[:, 0:HID],
                                        in1=v[:, HID:64],
                                        op=mybir.AluOpType.add)
                nc.scalar.mul(out=x3[:], in_=x3[:], mul=0.5)
                nc.vector.tensor_tensor(out=x3[:], in0=x3[:],
                                        in1=b2rep[:DW, :],
                                        op=mybir.AluOpType.add)
                m = sb.tile([DW, HID], F32, tag="m2")
                nc.vector.tensor_scalar_min(m[:], x3[:], 0.0)
                nc.scalar.activation(out=m[:], in_=m[:],
                                     func=mybir.ActivationFunctionType.Exp)
                nc.vector.tensor_scalar_sub(m[:], m[:], 1.0)
                x3b = sb.tile([DW, HID], BF, tag="x3b")
                nc.vector.tensor_tensor(out=x3b[:], in0=x3[:], in1=m[:],
                                        op=mybir.AluOpType.max)
                x3T_ps = ps.tile([HID, DW], BF, tag="sc")
                nc.tensor.transpose(out=x3T_ps[:], in_=x3b[:],
                                    identity=ident[:DW, :DW])
                x3T = sb.tile([HID, DW], BF, tag="x3T")
                nc.scalar.copy(out=x3T[:], in_=x3T_ps[:])
                y_ps = ps.tile([OUT_CH, DW], F32, tag="hg")
                nc.tensor.matmul(out=y_ps[:], lhsT=wlin_t[:], rhs=x3T[:],
                                 start=True, stop=True)
                ysb = sb.tile([OUT_CH, DW], F32, tag="ysb")
                nc.vector.tensor_scalar(out=ysb[:], in0=y_ps[:],
                                        scalar1=blin_t[:, :1], scalar2=None,
                                        op0=mybir.AluOpType.add)
                nc.sync.dma_start(out=yt[:, w * DW:(w + 1) * DW], in_=ysb[:])

            edge_layer(F1, s1t, d1t, l1t, t1g, t1d, H1, P, R1, epi1)

            nc.gpsimd.collective_compute(
                "AllGather", mybir.AluOpType.bypass,
                replica_groups=[list(range(NCORES))],
                ins=[t2loc[:]], outs=[t2full[:]])

            edge_layer(F2, s2t, d2t, l2t, t2full, t2loc, H2, 64, R2, epi2)

    nc.compile()
    return nc


def kernel(x, edge_index, W1, a_src1, a_dst1, b1, W2, a_src2, a_dst2, b2,
           W_lin, b_lin):
    x = np.asarray(x, np.float32)
    edge_index = np.asarray(edge_index)

    xpad = np.zeros((NPAD, IN_CH), np.float32)
    xpad[:N] = x
    xT_bf = np.ascontiguousarray(_bf(xpad).T)

    W1f = np.asarray(W1, np.float32)
    w1aug_bf = _bf(np.concatenate(
        [W1f, W1f @ _blockdiag(a_src1, H1, HID), W1f @ _blockdiag(a_dst1, H1, HID)],
        axis=1))
    W2f = np.asarray(W2, np.float32)
    w2aug_bf = _bf(np.concatenate(
        [W2f, W2f @ _blockdiag(a_src2, H2, HID), W2f @ _blockdiag(a_dst2, H2, HID)],
        axis=1))
    wlin_bf = _bf(W_lin)
    blin_col = np.asarray(b_lin, np.float32).reshape(OUT_CH, 1)
    b1_row = np.asarray(b1, np.float32).reshape(1, P)
    b2_row = np.asarray(b2, np.float32).reshape(1, HID)

    src = np.concatenate([edge_index[0].astype(np.int64),
                          np.arange(N, dtype=np.int64)])
    dst = np.concatenate([edge_index[1].astype(np.int64),
                          np.arange(N, dtype=np.int64)])

    # Degree-balanced window packing: renumber each core's local dsts
    # (degree-sorted, snake round-robin into the 98 windows) so the static
    # per-window chunk quota F tracks the average load, not the worst case.
    pc = []
    newlocals = []
    Fmax = 1
    for k in range(NCORES):
        m = (dst >= k * LOCAL) & (dst < (k + 1) * LOCAL)
        idx = np.nonzero(m)[0]
        dl_old = (dst[idx] - k * LOCAL).astype(np.int64)
        deg = np.bincount(dl_old, minlength=LOCAL)
        order = np.argsort(-deg, kind="stable")  # heavy dsts first
        i = np.arange(LOCAL)
        blk = i // NWIN
        win = np.where(blk % 2 == 0, i % NWIN, NWIN - 1 - (i % NWIN))
        slot = blk  # < DW since LOCAL/NWIN < 64
        newlocal = np.empty(LOCAL, np.int64)
        newlocal[order] = win * DW + slot
        newlocals.append(newlocal)
        dl = newlocal[dl_old]
        w = dl // DW
        Fmax = max(Fmax, int((np.bincount(w, minlength=NWIN).max() + P - 1) // P))
        pc.append((idx, dl, w))
    F1 = F2 = Fmax

    nc = _build_program(F1, F2)

    remap = np.zeros(N, np.int64)
    for k in range(NCORES):
        remap[k * LOCAL:(k + 1) * LOCAL] = 6272 * k + newlocals[k]
    s1_off = src * R1
    s2_off = remap[src] * R2
    in_maps = []
    for k in range(NCORES):
        idx, dl, w = pc[k]
        dl_full = np.zeros(len(dst), np.int64)
        dl_full[idx] = dl
        d1_off = dl_full * R1 + 132
        d2_off = dl_full * R2 + 64 + H2
        s1, d1, l1 = _tapes(pc[k], F1, s1_off, d1_off)
        s2, d2, l2 = _tapes(pc[k], F2, s2_off, d2_off)
        xd = np.zeros((LPAD, IN_CH), np.float32)
        xd[newlocals[k]] = x[k * LOCAL:(k + 1) * LOCAL]
        xdT_bf = np.ascontiguousarray(_bf(xd).T)
        in_maps.append({
            "xT": xT_bf, "xdT": xdT_bf, "w1aug": w1aug_bf, "w2aug": w2aug_bf,
            "wlin": wlin_bf, "blin": blin_col, "b1r": b1_row, "b2r": b2_row,
            "s1t": s1, "d1t": d1, "l1t": l1,
            "s2t": s2, "d2t": d2, "l2t": l2,
        })

    res = run_bass_kernel_spmd(nc, in_maps, list(range(NCORES)))
    out = np.zeros((N, OUT_CH), np.float32)
    for k in range(NCORES):
        ytk = res.results[k]["yt"]
        out[k * LOCAL:(k + 1) * LOCAL] = ytk[:, newlocals[k]].T
    return out

